# revision 10
# baseline (speedup 1.0000x reference)
"""Trainium2 Bass kernel for nn_DiffeqSolver: RK4 neural-ODE integration of
f(y) = conv2(tanh(conv1(y))), C=128, 3x3 SAME convs, data-parallel over
batch B=16 across 8 cores (2 images/core).

Speed tricks vs the fp16 9-tap baseline:
- fp8(e4m3) matmuls in DoubleRow perf mode: each PE pass contracts TWO conv
  taps (2 fp8 weights/cell), ~2x column throughput.
- Flat-window conv: activations live in a vertically-padded, 34-wide-row
  flat buffer, so every tap is a contiguous 1D window and any two taps form
  a legal [C, 2, N] DoubleRow rhs AP. Pair steps must be 16B-aligned, so a
  one-element-shifted duplicate of each buffer sits at an aligned distance
  (dup plane) for the delta=1 pairs; (0,2)+(1,0) pairs naturally (delta=32).
  Out-of-image columns in each 34-wide output row are garbage, never read.
- Weight quantization bias kill: W*64 = W8 + R8 (both e4m3, residual
  captures rounding), consumers scale psum by 1/64. 10 pair-matmuls/conv.
- RK4 with SKIP-sized steps + cubic-Hermite dense output (k-space
  accumulators) for skipped grid points; RK4 truncation error at dt<=0.16
  is orders below the fp8 noise floor.
"""
import sys

if '/opt/trn_rl_repo' not in sys.path:
    sys.path.insert(0, '/opt/trn_rl_repo')

import numpy as np
import ml_dtypes

import concourse.bass as bass
import concourse.tile as tile
from concourse import bacc, mybir
from concourse.bass_utils import run_bass_kernel_spmd

F32 = mybir.dt.float32
F8 = mybir.dt.float8e4
DR = mybir.MatmulPerfMode.DoubleRow
MULT = mybir.AluOpType.mult
ADD = mybir.AluOpType.add
Tanh = mybir.ActivationFunctionType.Tanh
Identity = mybir.ActivationFunctionType.Identity

B, C, H, W = 16, 128, 32, 32
NCORES = 8
IPC = B // NCORES
WP = 34                 # padded row width (1 + 32 + 1)
IMG = 35 * WP           # per-image flat span (1 top pad + 32 data + 2 bot pad)
NB = 2384               # flat buffer length (>= 1 + IPC*IMG + 1), 16-aligned
assert NB % 16 == 0 and NB >= 1 + IPC * IMG + 1
CHUNKS = [(0, 15), (15, 15), (30, 2)]
WSCALE = 64.0
# tap pairs: ((ky,kx),(ky,kx)|None). None = zero weights (2,2 partner).
# delta1 pairs use the dup plane (step NB); (0,2)+(1,0) pairs at step 32.
TPAIRS = [((0, 0), (0, 1)), ((0, 2), (1, 0)), ((1, 1), (1, 2)),
          ((2, 0), (2, 1)), ((2, 2), None)]
E4NP = ml_dtypes.float8_e4m3


def _off(b, r, kx):
    # flat index of window start for padded row r, tap kx (incl. -1 shift)
    return 1 + b * IMG + r * WP + kx - 1


def _build(dts, skip, b2_nonzero):
    nsteps = len(dts)          # fine steps (24)
    nbig = nsteps // skip
    hs = [float(np.sum(dts[n * skip:(n + 1) * skip])) for n in range(nbig)]
    nc = bacc.Bacc("TRN2", target_bir_lowering=False, debug=False,
                   num_devices=NCORES)

    x_d = nc.dram_tensor("x0", [C, IPC, H, W], F32, kind="ExternalInput")
    w_d = nc.dram_tensor("wall", [C, 40 * C], F32, kind="ExternalInput")
    b1_d = nc.dram_tensor("b1c", [C, 1], F32, kind="ExternalInput")
    b2_d = nc.dram_tensor("b2c", [C, 1], F32, kind="ExternalInput")
    out_d = nc.dram_tensor("out", [nsteps, C, IPC, H, W], F32,
                           kind="ExternalOutput")

    with tile.TileContext(nc) as tc:
        with (
            tc.tile_pool(name="persist", bufs=1) as pp,
            tc.tile_pool(name="psum", bufs=8, space="PSUM") as ps1,
            tc.tile_pool(name="outp", bufs=3) as op,
            tc.tile_pool(name="biasp", bufs=2) as bp,
        ):
            Y = pp.tile([C, IPC, H, W], F32, tag="Y")
            ACCS = [pp.tile([C, IPC, H, W], F32, tag=f"ACC{i}",
                            name=f"ACC{i}") for i in range(2)]
            K1SS = [pp.tile([C, IPC, H, W], F32, tag=f"K1S{i}",
                            name=f"K1S{i}") for i in range(2)]
            K4SS = [pp.tile([C, IPC, H, W], F32, tag=f"K4S{i}",
                            name=f"K4S{i}") for i in range(2)]
            # fp8 flat buffers, each with primary (plane 0) + shifted dup
            # (plane 1): FF[:,1,i] == FF[:,0,i+1]
            YB = pp.tile([C, 2, NB], F8, tag="YB")
            YT0 = pp.tile([C, 2, NB], F8, tag="YT0")
            YT1 = pp.tile([C, 2, NB], F8, tag="YT1")
            U0 = pp.tile([C, 2, NB], F8, tag="U0")
            U1 = pp.tile([C, 2, NB], F8, tag="U1")
            Wf = pp.tile([C, 40, C], F32, tag="Wf")
            Wq = pp.tile([C, 40, C], F8, tag="Wq")
            b1t = pp.tile([C, 1], F32, tag="b1t")
            b2t = pp.tile([C, 1], F32, tag="b2t")

            # PE warm-up during setup DMAs: garbage-input matmuls ramp the
            # HAM clock gate; outputs land in psum slots later start=True'd.
            warm = pp.tile([C, 5 * C], F8, tag="warm")
            nc.gpsimd.memset(warm[:], 0.0)
            for wi in range(24):
                pw = ps1.tile([C, 510], F32, tag="p", name=f"warm{wi}")
                nc.tensor.matmul(pw[:], warm[:, 0:C], warm[:, C:C + 510],
                                 start=True, stop=True)

            nc.sync.dma_start(Y[:], x_d[:])
            nc.sync.dma_start(Wf[:], w_d[:].rearrange("c (a o) -> c a o", a=40))
            nc.sync.dma_start(b1t[:], b1_d[:])
            nc.sync.dma_start(b2t[:], b2_d[:])

            # zero fp8 buffers (bit-zero == fp8 zero); convert weights
            for buf in (YB, YT0, YT1, U0, U1):
                nc.gpsimd.memset(buf[:], 0.0)
            nc.vector.tensor_copy(Wq[:], Wf[:])

            def interior(FF, plane, b, rows=None):
                """AP of data rows [r0, r0+nr) of image b in flat buffer
                plane (0 primary / 1 dup, dup shifted -1)."""
                r0, nr = rows if rows is not None else (0, H)
                base = 1 + b * IMG + (r0 + 1) * WP + 1 - (1 if plane else 0)
                a = FF[:, plane, 0:1].copy()
                a.offset += base
                a.ap[1] = [WP, nr]
                a = a.unsqueeze(2)
                a.ap[2] = [1, W]
                return a

            # initial YB from Y
            for b in range(IPC):
                nc.vector.tensor_copy(interior(YB, 0, b), Y[:, b])
                nc.scalar.activation(interior(YB, 1, b), Y[:, b], Identity)

            def rhs_ap(FF, b, r0, nr, pi):
                ta, tb = TPAIRS[pi]
                offA = _off(b, r0 + ta[0], ta[1])
                n = nr * WP
                a = FF[:, 0, offA:offA + n].unsqueeze(1)
                if tb is None:
                    a.ap[1] = [NB, 2]             # zero-weight half, dup plane
                else:
                    delta = _off(b, r0 + tb[0], tb[1]) - offA
                    if delta % 16 == 0:
                        a.ap[1] = [delta, 2]      # natural aligned pair
                    else:
                        assert delta == 1, (ta, tb)
                        a.ap[1] = [NB, 2]         # via shifted dup plane
                return a

            def conv(src, wsel, on_image, ps, ptag):
                """One conv, image-major: per image 10 DoubleRow
                pair-matmuls (5 W8 + 5 R8) into 3 chunk banks, then the
                per-image consumer callback (critical ops first)."""
                for b in range(IPC):
                    pts = [ps.tile([C, 512], F32, tag=ptag,
                                   name=f"{ptag}b{b}c{ci}")
                           for ci in range(3)]
                    for pu in range(10):
                        lhsT = Wq[:, (wsel * 10 + pu) * 2:
                                   (wsel * 10 + pu) * 2 + 2, :]
                        for ci, (r0, nr) in enumerate(CHUNKS):
                            nc.tensor.matmul(
                                pts[ci][:, 0:nr * WP], lhsT,
                                rhs_ap(src, b, r0, nr, pu % 5),
                                start=(pu == 0), stop=(pu == 9),
                                perf_mode=DR)
                    pvs = []
                    for ci, (r0, nr) in enumerate(CHUNKS):
                        pv = pts[ci][:, 0:1].copy()
                        pv.offset += 1
                        pv.ap[1] = [WP, nr]
                        pv = pv.unsqueeze(2)
                        pv.ap[2] = [1, W]
                        pvs.append((pv, (r0, nr)))
                    on_image(pvs, b)

            deferred = []
            dchunk = max(1, (2 * (skip - 1) + 3) // 4)
            for n in range(nbig):
                h = hs[n]
                ACC = ACCS[n % 2]
                K1S = K1SS[n % 2]
                K4S = K4SS[n % 2]
                pr_s = [h / 2, h / 2, h, None]
                aw = [h / 6, h / 3, h / 3, h / 6]
                # dense output (tail-space): oj = Ynew + (h01-1)*ACC
                #                                  + h*h10*k1 + h*h11*k4
                cjw = []
                for j in range(1, skip):
                    th = j / skip
                    h01 = -2 * th**3 + 3 * th**2
                    h10 = th**3 - 2 * th**2 + th
                    h11 = th**3 - th**2
                    cjw.append([h01 - 1.0, h * h10, h * h11])

                srcs = [YB, YT0, YT1, YT0]
                us = [U0, U1, U0, U1]
                for e in range(4):
                    ue = us[e]
                    dst = srcs[e + 1] if e < 3 else None

                    def tanh_image(pvs, b, ue=ue):
                        for pv, rows in pvs:
                            nc.scalar.activation(interior(ue, 0, b, rows),
                                                 pv, Tanh, bias=b1t[:, 0:1],
                                                 scale=1.0 / WSCALE)
                            nc.scalar.activation(interior(ue, 1, b, rows),
                                                 interior(ue, 0, b, rows),
                                                 Identity)

                    conv(srcs[e], 0, tanh_image, ps1, "p")
                    while deferred and len(deferred) > (3 - e) * dchunk:
                        deferred.pop(0)()

                    def k_image(pvs, b, e=e, n=n, h=h):
                        kins = []
                        for pv, rows in pvs:
                            kin, ksc = pv, 1.0 / WSCALE
                            if b2_nonzero:
                                pb = bp.tile([C, 15, W], F32, tag="pb",
                                             name=f"pb{n}{e}{b}{rows[0]}")
                                nc.scalar.activation(
                                    pb[:, 0:rows[1]], pv, Identity,
                                    bias=b2t[:, 0:1], scale=1.0 / WSCALE)
                                kin, ksc = pb[:, 0:rows[1]], 1.0
                            kins.append((kin, ksc, rows))
                        # critical path first: probe (+dup) feeds next conv1
                        if e < 3:
                            for kin, ksc, rows in kins:
                                r0, nr = rows
                                sl = (slice(None), b, slice(r0, r0 + nr))
                                nc.vector.scalar_tensor_tensor(
                                    interior(dst, 0, b, rows), kin,
                                    pr_s[e] * ksc, Y[sl], op0=MULT, op1=ADD)
                                nc.scalar.activation(
                                    interior(dst, 1, b, rows),
                                    interior(dst, 0, b, rows), Identity)
                        # accumulation; K-saves deferred past the unblock
                        for kin, ksc, rows in kins:
                            r0, nr = rows
                            sl = (slice(None), b, slice(r0, r0 + nr))
                            if e == 0:
                                nc.vector.tensor_scalar_mul(
                                    ACC[sl], kin, aw[0] * ksc)
                            else:
                                nc.vector.scalar_tensor_tensor(
                                    ACC[sl], kin, aw[e] * ksc, ACC[sl],
                                    op0=MULT, op1=ADD)
                        if e == 3:
                            # unblock next conv1: Y, YB, dup for THIS image
                            nc.vector.tensor_add(
                                Y[:, b], Y[:, b], ACC[:, b])
                            nc.vector.tensor_copy(interior(YB, 0, b),
                                                  Y[:, b])
                            nc.scalar.activation(interior(YB, 1, b),
                                                 interior(YB, 0, b),
                                                 Identity)
                            nc.sync.dma_start(
                                out_d[(n + 1) * skip - 1][:, b], Y[:, b])
                        if skip > 1 and e in (0, 3):
                            KS = K1S if e == 0 else K4S
                            for kin, ksc, rows in kins:
                                r0, nr = rows
                                sl = (slice(None), b, slice(r0, r0 + nr))
                                nc.vector.tensor_scalar_mul(
                                    KS[sl], kin, ksc)

                    conv(ue, 1, k_image, ps1, "p")

                # dense-output points: deferred — emitted in small
                # slices during the NEXT step's stages so they fill DVE
                # idle windows without delaying any probe op.
                for b in range(IPC):
                    for j in range(skip - 1):
                        def emit_oj(n=n, b=b, j=j, cj=cjw[j], ACC=ACC,
                                    K1S=K1S, K4S=K4S):
                            oj = op.tile([C, H, W], F32, tag=f"oj{j}",
                                         name=f"oj{j}_{n}_{b}")
                            nc.vector.scalar_tensor_tensor(
                                oj[:], ACC[:, b], cj[0], Y[:, b],
                                op0=MULT, op1=ADD)
                            nc.vector.scalar_tensor_tensor(
                                oj[:], K1S[:, b], cj[1], oj[:],
                                op0=MULT, op1=ADD)
                            nc.vector.scalar_tensor_tensor(
                                oj[:], K4S[:, b], cj[2], oj[:],
                                op0=MULT, op1=ADD)
                            nc.sync.dma_start(out_d[n * skip + j][:, b],
                                              oj[:])
                        deferred.append(emit_oj)

            for fn in deferred:
                fn()

    nc.compile()
    return nc


_CACHE = {}


def _get_program(dts, skip, b2_nonzero):
    key = (tuple(np.asarray(dts, dtype=np.float32).tolist()), skip, b2_nonzero)
    if key not in _CACHE:
        _CACHE[key] = _build(np.asarray(dts, dtype=np.float32), skip,
                             b2_nonzero)
    return _CACHE[key]


def _pack_weights(W1, W2):
    """[C,2,10,2,C]: dim1 conv idx; dim2 pair-unit (5 W8 + 5 R8);
    dim3 pair element."""
    wall = np.zeros((C, 2, 10, 2, C), np.float32)
    for wi, Wc in enumerate((W1, W2)):
        Ws = np.asarray(Wc, np.float32) * WSCALE      # [O,I,3,3]
        W8 = Ws.astype(E4NP).astype(np.float32)
        R8 = (Ws - W8).astype(E4NP).astype(np.float32)
        for pi, (ta, tb) in enumerate(TPAIRS):
            for half, src in ((0, W8), (1, R8)):
                wall[:, wi, pi + 5 * half, 0, :] = src[:, :, ta[0], ta[1]].T
                if tb is not None:
                    wall[:, wi, pi + 5 * half, 1, :] = \
                        src[:, :, tb[0], tb[1]].T
    return np.ascontiguousarray(wall.reshape(C, 40 * C))


def _run(first_point, time_steps_to_predict, W1, b1, W2, b2, trace=False):
    first_point = np.ascontiguousarray(first_point, dtype=np.float32)
    tgrid = np.asarray(time_steps_to_predict, dtype=np.float32)
    dts = np.diff(tgrid)
    nsteps = len(dts)
    b2 = np.asarray(b2, dtype=np.float32)
    b2_nonzero = bool(np.any(b2 != 0))

    skip = 1
    for cand in (6, 4, 3, 2):
        if nsteps % cand == 0 and np.ptp(dts) < 1e-6:
            skip = cand
            break

    nc = _get_program(dts, skip, b2_nonzero)

    wall = _pack_weights(W1, W2)
    b1c = np.ascontiguousarray(np.asarray(b1, np.float32).reshape(C, 1))
    b2c = np.ascontiguousarray(b2.reshape(C, 1))

    in_maps = []
    for i in range(NCORES):
        x0 = np.ascontiguousarray(
            first_point[IPC * i:IPC * (i + 1)].transpose(1, 0, 2, 3))
        in_maps.append({"x0": x0, "wall": wall, "b1c": b1c, "b2c": b2c})

    rr = run_bass_kernel_spmd(nc, in_maps, list(range(NCORES)), trace=trace)

    full = np.empty((B, nsteps + 1, C, H, W), dtype=np.float32)
    full[:, 0] = first_point
    for i in range(NCORES):
        o = rr.results[i]["out"]            # [nsteps, C, IPC, H, W]
        full[IPC * i:IPC * (i + 1), 1:] = o.transpose(2, 0, 1, 3, 4)
    return full, rr.exec_time_ns


def kernel(first_point, time_steps_to_predict, W1, b1, W2, b2):
    out, _ = _run(first_point, time_steps_to_predict, W1, b1, W2, b2)
    return out


# revision 12
# speedup vs baseline: 1.2692x; 1.2692x over previous
"""Trainium2 Bass kernel for nn_DiffeqSolver: RK4 neural-ODE integration of
f(y) = conv2(tanh(conv1(y))), C=128, 3x3 SAME convs, data-parallel over
batch B=16 across 8 cores (2 images/core).

Speed tricks vs the fp16 9-tap baseline:
- fp8(e4m3) matmuls in DoubleRow perf mode: each PE pass contracts TWO conv
  taps (2 fp8 weights/cell), ~2x column throughput.
- Flat-window conv: activations live in a vertically-padded, 34-wide-row
  flat buffer, so every tap is a contiguous 1D window and any two taps form
  a legal [C, 2, N] DoubleRow rhs AP. Pair steps must be 16B-aligned, so a
  one-element-shifted duplicate of each buffer sits at an aligned distance
  (dup plane) for the delta=1 pairs; (0,2)+(1,0) pairs naturally (delta=32).
  Out-of-image columns in each 34-wide output row are garbage, never read.
- Weight quantization bias kill: W*64 = W8 + R8 (both e4m3, residual
  captures rounding), consumers scale psum by 1/64. 10 pair-matmuls/conv.
- RK4 with SKIP-sized steps + cubic-Hermite dense output (k-space
  accumulators) for skipped grid points; RK4 truncation error at dt<=0.16
  is orders below the fp8 noise floor.
"""
import sys

if '/opt/trn_rl_repo' not in sys.path:
    sys.path.insert(0, '/opt/trn_rl_repo')

import numpy as np
import ml_dtypes

import concourse.bass as bass
import concourse.tile as tile
from concourse import bacc, mybir
from concourse.bass_utils import run_bass_kernel_spmd

F32 = mybir.dt.float32
F8 = mybir.dt.float8e4
DR = mybir.MatmulPerfMode.DoubleRow
MULT = mybir.AluOpType.mult
ADD = mybir.AluOpType.add
Tanh = mybir.ActivationFunctionType.Tanh
Identity = mybir.ActivationFunctionType.Identity

B, C, H, W = 16, 128, 32, 32
NCORES = 8
IPC = B // NCORES
WP = 34                 # padded row width (1 + 32 + 1)
IMG = 35 * WP           # per-image flat span (1 top pad + 32 data + 2 bot pad)
NB = 2384               # flat buffer length (>= 1 + IPC*IMG + 1), 16-aligned
assert NB % 16 == 0 and NB >= 1 + IPC * IMG + 1
CHUNKS = [(0, 15), (15, 15), (30, 2)]
WSCALE = 64.0
# tap pairs: ((ky,kx),(ky,kx)|None). None = zero weights (2,2 partner).
# delta1 pairs use the dup plane (step NB); (0,2)+(1,0) pairs at step 32.
TPAIRS = [((0, 0), (0, 1)), ((0, 2), (1, 0)), ((1, 1), (1, 2)),
          ((2, 0), (2, 1)), ((2, 2), None)]
E4NP = ml_dtypes.float8_e4m3


def _off(b, r, kx):
    # flat index of window start for padded row r, tap kx (incl. -1 shift)
    return 1 + b * IMG + r * WP + kx - 1


def _build(dts, skip, b2_nonzero):
    nsteps = len(dts)          # fine steps (24)
    nbig = nsteps // skip
    hs = [float(np.sum(dts[n * skip:(n + 1) * skip])) for n in range(nbig)]
    nc = bacc.Bacc("TRN2", target_bir_lowering=False, debug=False,
                   num_devices=NCORES)

    x_d = nc.dram_tensor("x0", [C, IPC, H, W], F32, kind="ExternalInput")
    w_d = nc.dram_tensor("wall", [C, 40 * C], F32, kind="ExternalInput")
    b1_d = nc.dram_tensor("b1c", [C, 1], F32, kind="ExternalInput")
    b2_d = nc.dram_tensor("b2c", [C, 1], F32, kind="ExternalInput")
    out_d = nc.dram_tensor("out", [nsteps, C, IPC, H, W], F32,
                           kind="ExternalOutput")

    with tile.TileContext(nc) as tc:
        with (
            tc.tile_pool(name="persist", bufs=1) as pp,
            tc.tile_pool(name="psum", bufs=8, space="PSUM") as ps1,
            tc.tile_pool(name="outp", bufs=3) as op,
            tc.tile_pool(name="biasp", bufs=2) as bp,
        ):
            Y = pp.tile([C, IPC, H, W], F32, tag="Y")
            ACC = pp.tile([C, IPC, H, W], F32, tag="ACC")
            K1S = pp.tile([C, IPC, H, W], F32, tag="K1S")
            K4S = pp.tile([C, IPC, H, W], F32, tag="K4S")
            # fp8 flat buffers, each with primary (plane 0) + shifted dup
            # (plane 1): FF[:,1,i] == FF[:,0,i+1]
            YB = pp.tile([C, 2, NB], F8, tag="YB")
            YT0 = pp.tile([C, 2, NB], F8, tag="YT0")
            YT1 = pp.tile([C, 2, NB], F8, tag="YT1")
            U0 = pp.tile([C, 2, NB], F8, tag="U0")
            U1 = pp.tile([C, 2, NB], F8, tag="U1")
            Wf = pp.tile([C, 40, C], F32, tag="Wf")
            Wq = pp.tile([C, 40, C], F8, tag="Wq")
            b1t = pp.tile([C, 1], F32, tag="b1t")
            b2t = pp.tile([C, 1], F32, tag="b2t")

            # PE warm-up during setup DMAs: garbage-input matmuls ramp the
            # HAM clock gate; outputs land in psum slots later start=True'd.
            warm = pp.tile([C, 5 * C], F8, tag="warm")
            nc.gpsimd.memset(warm[:], 0.0)
            for wi in range(24):
                pw = ps1.tile([C, 510], F32, tag="p", name=f"warm{wi}")
                nc.tensor.matmul(pw[:], warm[:, 0:C], warm[:, C:C + 510],
                                 start=True, stop=True)

            nc.sync.dma_start(Y[:], x_d[:])
            nc.sync.dma_start(Wf[:], w_d[:].rearrange("c (a o) -> c a o", a=40))
            nc.sync.dma_start(b1t[:], b1_d[:])
            nc.sync.dma_start(b2t[:], b2_d[:])

            # zero fp8 buffers (bit-zero == fp8 zero); convert weights
            for buf in (YB, YT0, YT1, U0, U1):
                nc.gpsimd.memset(buf[:], 0.0)
            nc.vector.tensor_copy(Wq[:], Wf[:])

            def interior(FF, plane, b, rows=None):
                """AP of data rows [r0, r0+nr) of image b in flat buffer
                plane (0 primary / 1 dup, dup shifted -1)."""
                r0, nr = rows if rows is not None else (0, H)
                base = 1 + b * IMG + (r0 + 1) * WP + 1 - (1 if plane else 0)
                a = FF[:, plane, 0:1].copy()
                a.offset += base
                a.ap[1] = [WP, nr]
                a = a.unsqueeze(2)
                a.ap[2] = [1, W]
                return a

            # initial YB from Y
            for b in range(IPC):
                nc.vector.tensor_copy(interior(YB, 0, b), Y[:, b])
                nc.scalar.activation(interior(YB, 1, b), Y[:, b], Identity)

            def rhs_ap(FF, b, r0, nr, pi):
                ta, tb = TPAIRS[pi]
                offA = _off(b, r0 + ta[0], ta[1])
                n = nr * WP
                a = FF[:, 0, offA:offA + n].unsqueeze(1)
                if tb is None:
                    a.ap[1] = [NB, 2]             # zero-weight half, dup plane
                else:
                    delta = _off(b, r0 + tb[0], tb[1]) - offA
                    if delta % 16 == 0:
                        a.ap[1] = [delta, 2]      # natural aligned pair
                    else:
                        assert delta == 1, (ta, tb)
                        a.ap[1] = [NB, 2]         # via shifted dup plane
                return a

            def conv(src, wsel, on_image, ps, ptag):
                """One conv, image-major: per image 10 DoubleRow
                pair-matmuls (5 W8 + 5 R8) into 3 chunk banks, then the
                per-image consumer callback (critical ops first)."""
                for b in range(IPC):
                    pts = [ps.tile([C, 512], F32, tag=ptag,
                                   name=f"{ptag}b{b}c{ci}")
                           for ci in range(3)]
                    for pu in range(10):
                        lhsT = Wq[:, (wsel * 10 + pu) * 2:
                                   (wsel * 10 + pu) * 2 + 2, :]
                        for ci, (r0, nr) in enumerate(CHUNKS):
                            nc.tensor.matmul(
                                pts[ci][:, 0:nr * WP], lhsT,
                                rhs_ap(src, b, r0, nr, pu % 5),
                                start=(pu == 0), stop=(pu == 9),
                                perf_mode=DR)
                    pvs = []
                    for ci, (r0, nr) in enumerate(CHUNKS):
                        pv = pts[ci][:, 0:1].copy()
                        pv.offset += 1
                        pv.ap[1] = [WP, nr]
                        pv = pv.unsqueeze(2)
                        pv.ap[2] = [1, W]
                        pvs.append((pv, (r0, nr)))
                    on_image(pvs, b)

            for n in range(nbig):
                h = hs[n]
                pr_s = [h / 2, h / 2, h, None]
                aw = [h / 6, h / 3, h / 3, h / 6]
                # dense output (tail-space): oj = Ynew + (h01-1)*ACC
                #                                  + h*h10*k1 + h*h11*k4
                cjw = []
                for j in range(1, skip):
                    th = j / skip
                    h01 = -2 * th**3 + 3 * th**2
                    h10 = th**3 - 2 * th**2 + th
                    h11 = th**3 - th**2
                    cjw.append([h01 - 1.0, h * h10, h * h11])

                srcs = [YB, YT0, YT1, YT0]
                us = [U0, U1, U0, U1]
                for e in range(4):
                    ue = us[e]
                    dst = srcs[e + 1] if e < 3 else None

                    def tanh_image(pvs, b, ue=ue):
                        for pv, rows in pvs:
                            nc.scalar.activation(interior(ue, 0, b, rows),
                                                 pv, Tanh, bias=b1t[:, 0:1],
                                                 scale=1.0 / WSCALE)
                            nc.scalar.activation(interior(ue, 1, b, rows),
                                                 interior(ue, 0, b, rows),
                                                 Identity)

                    conv(srcs[e], 0, tanh_image, ps1, "p")

                    def k_image(pvs, b, e=e, n=n, h=h):
                        kins = []
                        for pv, rows in pvs:
                            kin, ksc = pv, 1.0 / WSCALE
                            if b2_nonzero:
                                pb = bp.tile([C, 15, W], F32, tag="pb",
                                             name=f"pb{n}{e}{b}{rows[0]}")
                                nc.scalar.activation(
                                    pb[:, 0:rows[1]], pv, Identity,
                                    bias=b2t[:, 0:1], scale=1.0 / WSCALE)
                                kin, ksc = pb[:, 0:rows[1]], 1.0
                            kins.append((kin, ksc, rows))
                        # critical path first: probe (+dup) feeds next conv1
                        if e < 3:
                            for kin, ksc, rows in kins:
                                r0, nr = rows
                                sl = (slice(None), b, slice(r0, r0 + nr))
                                nc.vector.scalar_tensor_tensor(
                                    interior(dst, 0, b, rows), kin,
                                    pr_s[e] * ksc, Y[sl], op0=MULT, op1=ADD)
                                nc.scalar.activation(
                                    interior(dst, 1, b, rows),
                                    interior(dst, 0, b, rows), Identity)
                        # lag-tolerant accumulation
                        for kin, ksc, rows in kins:
                            r0, nr = rows
                            sl = (slice(None), b, slice(r0, r0 + nr))
                            if e == 0:
                                nc.vector.tensor_scalar_mul(
                                    ACC[sl], kin, aw[0] * ksc)
                                if skip > 1:
                                    nc.vector.tensor_scalar_mul(
                                        K1S[sl], kin, ksc)
                            else:
                                nc.vector.scalar_tensor_tensor(
                                    ACC[sl], kin, aw[e] * ksc, ACC[sl],
                                    op0=MULT, op1=ADD)
                                if e == 3 and skip > 1:
                                    nc.vector.tensor_scalar_mul(
                                        K4S[sl], kin, ksc)
                        if e == 3:
                            # unblock next conv1: Y, YB, dup for THIS image
                            nc.vector.tensor_add(
                                Y[:, b], Y[:, b], ACC[:, b])
                            nc.vector.tensor_copy(interior(YB, 0, b),
                                                  Y[:, b])
                            nc.scalar.activation(interior(YB, 1, b),
                                                 interior(YB, 0, b),
                                                 Identity)
                            nc.sync.dma_start(
                                out_d[(n + 1) * skip - 1][:, b], Y[:, b])

                    conv(ue, 1, k_image, ps1, "p")

                # dense-output points: off the critical path, overlap with
                # the next step's matmuls
                for b in range(IPC):
                    for j in range(skip - 1):
                        oj = op.tile([C, H, W], F32, tag=f"oj{j}",
                                     name=f"oj{j}_{n}_{b}")
                        nc.vector.scalar_tensor_tensor(
                            oj[:], ACC[:, b], cjw[j][0], Y[:, b],
                            op0=MULT, op1=ADD)
                        nc.vector.scalar_tensor_tensor(
                            oj[:], K1S[:, b], cjw[j][1], oj[:],
                            op0=MULT, op1=ADD)
                        nc.vector.scalar_tensor_tensor(
                            oj[:], K4S[:, b], cjw[j][2], oj[:],
                            op0=MULT, op1=ADD)
                        nc.sync.dma_start(out_d[n * skip + j][:, b], oj[:])

    nc.compile()
    return nc


_CACHE = {}


def _get_program(dts, skip, b2_nonzero):
    key = (tuple(np.asarray(dts, dtype=np.float32).tolist()), skip, b2_nonzero)
    if key not in _CACHE:
        _CACHE[key] = _build(np.asarray(dts, dtype=np.float32), skip,
                             b2_nonzero)
    return _CACHE[key]


def _pack_weights(W1, W2):
    """[C,2,10,2,C]: dim1 conv idx; dim2 pair-unit (5 W8 + 5 R8);
    dim3 pair element."""
    wall = np.zeros((C, 2, 10, 2, C), np.float32)
    for wi, Wc in enumerate((W1, W2)):
        Ws = np.asarray(Wc, np.float32) * WSCALE      # [O,I,3,3]
        W8 = Ws.astype(E4NP).astype(np.float32)
        R8 = (Ws - W8).astype(E4NP).astype(np.float32)
        for pi, (ta, tb) in enumerate(TPAIRS):
            for half, src in ((0, W8), (1, R8)):
                wall[:, wi, pi + 5 * half, 0, :] = src[:, :, ta[0], ta[1]].T
                if tb is not None:
                    wall[:, wi, pi + 5 * half, 1, :] = \
                        src[:, :, tb[0], tb[1]].T
    return np.ascontiguousarray(wall.reshape(C, 40 * C))


def _run(first_point, time_steps_to_predict, W1, b1, W2, b2, trace=False):
    first_point = np.ascontiguousarray(first_point, dtype=np.float32)
    tgrid = np.asarray(time_steps_to_predict, dtype=np.float32)
    dts = np.diff(tgrid)
    nsteps = len(dts)
    b2 = np.asarray(b2, dtype=np.float32)
    b2_nonzero = bool(np.any(b2 != 0))

    skip = 1
    for cand in (8, 6, 4, 3, 2):
        if nsteps % cand == 0 and np.ptp(dts) < 1e-6:
            skip = cand
            break

    nc = _get_program(dts, skip, b2_nonzero)

    wall = _pack_weights(W1, W2)
    b1c = np.ascontiguousarray(np.asarray(b1, np.float32).reshape(C, 1))
    b2c = np.ascontiguousarray(b2.reshape(C, 1))

    in_maps = []
    for i in range(NCORES):
        x0 = np.ascontiguousarray(
            first_point[IPC * i:IPC * (i + 1)].transpose(1, 0, 2, 3))
        in_maps.append({"x0": x0, "wall": wall, "b1c": b1c, "b2c": b2c})

    rr = run_bass_kernel_spmd(nc, in_maps, list(range(NCORES)), trace=trace)

    full = np.empty((B, nsteps + 1, C, H, W), dtype=np.float32)
    full[:, 0] = first_point
    for i in range(NCORES):
        o = rr.results[i]["out"]            # [nsteps, C, IPC, H, W]
        full[IPC * i:IPC * (i + 1), 1:] = o.transpose(2, 0, 1, 3, 4)
    return full, rr.exec_time_ns


def kernel(first_point, time_steps_to_predict, W1, b1, W2, b2):
    out, _ = _run(first_point, time_steps_to_predict, W1, b1, W2, b2)
    return out
